# revision 85
# baseline (speedup 1.0000x reference)
"""JPEG encoder Bass kernel for TRN2 — self-contained, 8-core data-parallel.

kernel(img, D, Q) -> (flatten, no_quan_flatten), matching the reference:
    per 8x8 block: dct = D @ (X - 128) @ D.T ; quant = round(dct / Q);
    both zigzag-gathered + channel-concatenated to (256, 512, 192).

Device computes ONLY the unquantized DCT as int16 = round(16 * dct),
with X-128 pre-centered on the host (halves fp16 rounding noise; the
scale 16 is folded into the weights and |16*dct(X-128)| <= 16384 always
fits int16). The host derives nq = int16/16 and quant = round(nq/Q).
Dropping the on-device int8 quant stream cuts output DMA by 1/3 and
deletes the entire DVE quant pipeline.

Per 2-block-row strip (128 partitions = (brp, b), free = (c, i, w)):
  input DMA (2KB runs, SP queue) -> DVE regroup into z-grouped layout
  (c01 via two 4x-mode strided copies; c2 on gpsimd) -> 24 PE transposes
  in three 8-chunk units (c01 q0-3 | c2 all-q | c01 q4-7), each filling
  exactly one PSUM bank f16 -> DVE 2x copies to SBUF -> 24 single-shot
  fp16 matmuls against one block-diag(16*kron(D,D)[zz].T) weight
  (per-block c01 128-wide + merged c2 pair via the same block-diag) ->
  f32 PSUM half-strip tiles (3 banks, ring 2) -> Act f32->int16 cast
  copies (RNE) -> one whole-strip SWDGE out-DMA on the gpsimd queue
  (994ns fixed cost paid once; 6KB runs).

Schedule notes (cost-model driven): transposes are emitted two units
ahead (px ring 2 = 2 banks; po ring 2 = 6; PSUM exactly 8 banks); c01
matmuls are emitted a unit before the c2 matmuls that complete each po
half so the Act outcopy chain stays hot; DMA-issue SEQ-holds are kept
off the Act/DVE queues (inputs on SP, outputs on gpsimd/SWDGE whose
waits queue like normal engine ops); the last strip splits outcopies
Act||DVE and out-DMAs across two queues to shorten the drain. DMA floor
is ~35.3us/core (6.3MB fp16 in + 6.3MB int16 out at 360 GB/s).
"""

import numpy as np
import concourse.mybir as mybir
import concourse.tile as tile
from concourse import bacc
from concourse.bass_utils import run_bass_kernel_spmd

F32 = mybir.dt.float32
F16 = mybir.dt.float16
I16 = mybir.dt.int16
P = 8
B, C, H, W = 512, 3, 128, 128
NCORES = 8
BSH = B // NCORES          # 64 batches per core
N = (H // P) * (W // P)    # 256 blocks per plane
CZ = C * P * P             # 192
NBR = H // P               # 16 block rows
NSTRIP = NBR // 2          # 8 strips of 2 block rows

Copy = mybir.ActivationFunctionType.Copy


def _zigzag_flat_idx(n=P):
    order = []
    for s in range(2 * n - 1):
        cells = [(r, s - r) for r in range(max(0, s - n + 1), min(s, n - 1) + 1)]
        if s % 2 == 0:
            cells.reverse()
        order.extend(cells)
    return np.array([r * n + c for r, c in order], dtype=np.int32)


def _build_consts(D: np.ndarray):
    ZZ = _zigzag_flat_idx()
    D64 = D.astype(np.float64)
    KD = np.kron(D64, D64)[ZZ, :]          # (64 zz, 64 pix)
    Mt = (16.0 * KD.T).astype(np.float16)  # (pix, zz), x16 output scale
    Wm = np.zeros((128, 128), dtype=np.float16)
    Wm[0:64, 0:64] = Mt
    Wm[64:128, 64:128] = Mt
    bdid = np.eye(128, dtype=np.float16)
    return Wm, bdid


def _build_nc():
    nc = bacc.Bacc("TRN2", target_bir_lowering=False, debug=False)

    img = nc.dram_tensor("img", [BSH, C, H, W], F16, kind="ExternalInput")
    W_d = nc.dram_tensor("Wm", [128, 128], F16, kind="ExternalInput")
    # device layout: [strip, brp, b, (Q, c01|c2 packed)]; host reassembles
    out_d = nc.dram_tensor("out", [NSTRIP, 2, BSH, 3072], I16,
                           kind="ExternalOutput")

    # partition = (brp, b), free = (c, (i, w)) with 2KB contiguous runs
    imgv = img[:].rearrange("b c (bp brp i) w -> bp brp b c (i w)",
                            brp=2, i=P)

    with tile.TileContext(nc) as tc:
        with (
            tc.tile_pool(name="const", bufs=1) as constp,
            tc.tile_pool(name="sload", bufs=4) as sload,
            tc.tile_pool(name="greg", bufs=4) as greg,
            tc.tile_pool(name="x", bufs=6) as xp,
            tc.tile_pool(name="sb", bufs=5) as sbp,
            tc.tile_pool(name="sbl", bufs=4) as sblp,
            tc.tile_pool(name="px", bufs=2, space="PSUM") as pxp,
            tc.tile_pool(name="po", bufs=2, space="PSUM") as pop,
        ):
            Wm = constp.tile([128, 128], F16)
            bdid = constp.tile([128, 128], F16)
            ones = constp.tile([128, 128], F16)

            S_t = [None] * NSTRIP
            G_t = [None] * NSTRIP
            x_t = [None] * (NSTRIP * 4)
            px_t = [None] * (NSTRIP * 4)
            po_t = [None] * (NSTRIP * 2)
            sb_t = [None] * NSTRIP
            sbl_t = [None] * (NSTRIP * 2)

            def load_s(bp, split=False):
                S = sload.tile([128, 3072], F16, tag="s", name=f"s{bp}")
                if split:
                    # per-c-plane DMAs on three queues so the regroup
                    # copies can start as early as possible
                    Sv3 = S[:].rearrange("p (c iw) -> p c iw", c=3, iw=1024)
                    nc.sync.dma_start(out=Sv3[:, 0], in_=imgv[bp, :, :, 0])
                    nc.gpsimd.dma_start(out=Sv3[:, 1], in_=imgv[bp, :, :, 1])
                    nc.sync.dma_start(out=Sv3[:, 2], in_=imgv[bp, :, :, 2])
                else:
                    # per-plane DMAs: finer DMA_ENGINES granularity lets
                    # ready outputs interleave sooner than 2185ns strips
                    Sv3 = S[:].rearrange("p (c iw) -> p c iw", c=3, iw=1024)
                    for c in range(3):
                        nc.sync.dma_start(out=Sv3[:, c],
                                          in_=imgv[bp, :, :, c])
                S_t[bp] = S

            def regroup(bp, part=None):
                # S free (c,i,q,k,j) -> G (q, z, i, j); z = 2k+c | 4+k
                # part 0/1: c01 k-halves; part 2: c2 (emitted first).
                if part in (2, None):
                    G_t[bp] = greg.tile([128, 3072], F16, tag="g",
                                        name=f"g{bp}")
                G = G_t[bp]
                gv = G[:].rearrange("p (q z i j) -> p z q i j",
                                    q=8, z=6, i=P, j=P)
                sv = S_t[bp][:].rearrange("p (c i q k j) -> p k c q i j",
                                          c=3, i=P, q=8, k=2, j=P)
                parts = range(3) if part is None else [part]
                for pt in parts:
                    if pt == 1:
                        nc.vector.tensor_copy(gv[:, 2:4], sv[:, 1, 0:2])
                    elif pt == 0 and part is None:
                        nc.vector.tensor_copy(gv[:, 0:2], sv[:, 0, 0:2])
                    elif pt == 0:
                        # A on gpsimd too: keeps the DVE xcopy chain clean
                        nc.gpsimd.tensor_copy(gv[:, 0:2], sv[:, 0, 0:2])
                    elif part is None:
                        nc.vector.tensor_copy(gv[:, 4:6], sv[:, :, 2])
                    else:
                        # c2 regroup on gpsimd, emitted ahead of this
                        # strip's out-DMAs in the gpsimd queue
                        nc.gpsimd.tensor_copy(gv[:, 4:6], sv[:, :, 2])

            def transposes(v):
                # unit a (uu=0): c01 of q0-3; unit b (uu=1): c2 of all q;
                # unit c (uu=2): c01 of q4-7. Each 8 chunks = 1 PSUM bank.
                bp, uu = v // 3, v % 3
                G = G_t[bp]
                px = pxp.tile([128, 1024], F16, tag="px", name=f"px{v}")
                if uu == 1:
                    for q in range(8):
                        nc.tensor.matmul(
                            px[:, q * 128:(q + 1) * 128],
                            G[:, q * 384 + 256:q * 384 + 384],
                            bdid[:], is_transpose=True)
                else:
                    q0 = 0 if uu == 0 else 4
                    for t in range(4):
                        q = q0 + t
                        for k in range(2):
                            c = t * 2 + k
                            nc.tensor.matmul(
                                px[:, c * 128:(c + 1) * 128],
                                G[:, q * 384 + k * 128:
                                  q * 384 + (k + 1) * 128],
                                bdid[:], is_transpose=True)
                px_t[v] = px

            def xcopy(v):
                uu = v % 3
                x = xp.tile([128, 1024], F16, tag="x", name=f"x{v}")
                if uu == 1:
                    # split so M(h0) waits only the first c2 half
                    nc.vector.tensor_copy(x[:, 0:512], px_t[v][:, 0:512])
                    nc.vector.tensor_copy(x[:, 512:1024],
                                          px_t[v][:, 512:1024])
                else:
                    nc.vector.tensor_copy(x[:], px_t[v][:])
                x_t[v] = x

            def mm_c01(h):
                # half-strip po tile: [c01 8x128 | c2 4x128] per half
                bp, hh = h // 2, h % 2
                xa = x_t[bp * 3 + (0 if hh == 0 else 2)]
                if hh == 0:
                    po_t[h] = pop.tile([128, 1536], F32, tag="po",
                                       name=f"po{h}")
                po = po_t[h]
                for t in range(8):
                    nc.tensor.matmul(po[:, t * 128:(t + 1) * 128],
                                     xa[:, t * 128:(t + 1) * 128], Wm[:],
                                     start=True, stop=True)

            def mm_c2(h):
                bp, hh = h // 2, h % 2
                xb = x_t[bp * 3 + 1]
                if hh == 1:
                    po_t[h] = pop.tile([128, 1536], F32, tag="po",
                                       name=f"po{h}")
                po = po_t[h]
                for p in range(4):
                    nc.tensor.matmul(
                        po[:, 1024 + p * 128:1024 + (p + 1) * 128],
                        xb[:, (hh * 4 + p) * 128:(hh * 4 + p + 1) * 128],
                        Wm[:], start=True, stop=True)

            def outcopy(h, last=False):
                bp, hh = h // 2, h % 2
                if hh == 0 and not last:
                    sb_t[bp] = sbp.tile([128, 3072], I16, tag="sb",
                                        name=f"sb{bp}")
                if last:
                    # last strip: separate tiles per piece so the Act/DVE
                    # split copies aren't serialized by same-tile WAW deps
                    ta = sblp.tile([128, 768], I16, tag="sbl",
                                   name=f"sbl{h}a")
                    tb = sblp.tile([128, 768], I16, tag="sbl",
                                   name=f"sbl{h}b")
                    nc.scalar.activation(ta[:], po_t[h][:, 0:768], Copy)
                    nc.vector.tensor_copy(tb[:], po_t[h][:, 768:1536])
                    sbl_t[h] = (ta, tb)
                    return
                dst = sb_t[bp][:, hh * 1536:(hh + 1) * 1536]
                nc.scalar.activation(dst, po_t[h][:], Copy)

            def out_dma(bp, half, last=False):
                base = half * 1536
                if last:
                    # last strip: two parallel-queue DMAs to shorten tail
                    ta, tb = sbl_t[bp * 2 + half]
                    q2 = nc.sync if half else nc.scalar
                    nc.gpsimd.dma_start(
                        out=out_d[bp, :, :, base:base + 768], in_=ta[:])
                    q2.dma_start(
                        out=out_d[bp, :, :, base + 768:base + 1536],
                        in_=tb[:])
                    return
                fsl = slice(base, base + 1536)
                nc.gpsimd.dma_start(out=out_d[bp, :, :, fsl],
                                    in_=sb_t[bp][:, fsl])

            # ---- software-pipelined emission over 24 units ----
            # build the transpose identity on-device (memset + affine
            # select on iota p-f == 0): ready ~0.5us, no DMA in the lead
            # FIFO; Wm rides the SWDGE queue so it can't delay the planes
            load_s(0, split=True)
            nc.gpsimd.memset(ones[:], 1.0)
            nc.gpsimd.affine_select(bdid[:], ones[:], pattern=[[-1, 128]],
                                    compare_op=mybir.AluOpType.is_equal,
                                    fill=0.0, base=0, channel_multiplier=1)
            nc.gpsimd.dma_start(out=Wm[:], in_=W_d[:])
            regroup(0)
            load_s(1)
            # PE p-state warmup: dummy transposes on bdid bridge the gap
            # until the first real transpose so the 3us ramp to full clock
            # starts at ~3us instead of ~5us
            warm = pxp.tile([128, 1024], F16, tag="px", name="warm")
            for w in range(16):
                nc.tensor.matmul(warm[:, (w % 8) * 128:(w % 8 + 1) * 128],
                                 bdid[:], bdid[:], is_transpose=True)
            transposes(0)
            transposes(1)
            NV = NSTRIP * 3
            for v in range(NV):
                bp, uu = v // 3, v % 3
                if uu == 2 and bp + 2 < NSTRIP:
                    load_s(bp + 2)
                xcopy(v)
                if bp + 1 < NSTRIP:
                    # uu0 -> c2 on gpsimd (ahead of this strip's out-DMAs
                    # in the gpsimd queue) + c01-k0; uu1 -> c01-k1
                    if uu == 0:
                        regroup(bp + 1, part=2)
                        regroup(bp + 1, part=0)
                    elif uu == 1:
                        regroup(bp + 1, part=1)
                if uu == 0:
                    mm_c01(bp * 2)
                elif uu == 1:
                    mm_c2(bp * 2)
                else:
                    mm_c01(bp * 2 + 1)
                if v + 2 < NV:
                    transposes(v + 2)
                if uu == 1:
                    # po(h1) alloc waits the ring; emit after T so the
                    # wait can't head-of-line-block the transposes
                    mm_c2(bp * 2 + 1)
                if uu > 0:
                    h = bp * 2 + uu - 1
                    last = bp == NSTRIP - 1
                    outcopy(h, last=last)
                    # one whole-strip out-DMA (halves the SWDGE fixed
                    # cost); the last strip keeps per-half split DMAs
                    if last:
                        out_dma(bp, h % 2, last=True)
                    elif bp >= NSTRIP - 3:
                        # drain phase: per-half DMAs so h0 transfers early
                        out_dma(bp, h % 2)
                    elif uu == 2:
                        nc.gpsimd.dma_start(out=out_d[bp],
                                            in_=sb_t[bp][:])

    nc.compile()
    return nc


_NC_CACHE = None


def _get_nc():
    global _NC_CACHE
    if _NC_CACHE is None:
        _NC_CACHE = _build_nc()
    return _NC_CACHE


def _build_perm():
    # device free offset for (bw, czz): half h = bw//8, local bwl = bw%8;
    # c01 at h*1536 + bwl*128 + c*64 + zz; c2 at h*1536 + 1024 + bwl*64 + zz
    perm = np.zeros(NBR * CZ, dtype=np.int64)
    for bw in range(16):
        h, bwl = bw // 8, bw % 8
        for c in range(3):
            for zz in range(64):
                col = bw * CZ + c * 64 + zz
                if c < 2:
                    off = h * 1536 + bwl * 128 + c * 64 + zz
                else:
                    off = h * 1536 + 1024 + bwl * 64 + zz
                perm[col] = off
    return perm


_PERM = _build_perm()


def kernel(img, D, Q):
    img = np.asarray(img, dtype=np.float32)
    D = np.asarray(D, dtype=np.float32)
    Q = np.asarray(Q, dtype=np.float32)
    Wm, _ = _build_consts(D)
    ZZ = _zigzag_flat_idx()
    q_zz = np.tile(Q.flatten()[ZZ], C).astype(np.float32)     # (192,)

    # subtract 128 on host: halves fp16 input/weight noise and keeps
    # |16*dct| <= 16384 in int16 (dct of X-128 matches the reference)
    img16 = np.ascontiguousarray(img - np.float32(128.0)).astype(np.float16)
    nc = _get_nc()
    in_maps = [
        {"img": img16[kk * BSH:(kk + 1) * BSH], "Wm": Wm}
        for kk in range(NCORES)
    ]
    res = run_bass_kernel_spmd(nc, in_maps, core_ids=list(range(NCORES)))

    parts = []
    for r in res.results:
        dev = np.asarray(r["out"])                 # (8, 2, 64, 3072) i16
        f = dev[..., _PERM].astype(np.float32)     # (8, 2, 64, 16*192)
        f = f.reshape(NSTRIP, 2, BSH, NBR, CZ)
        f = f.transpose(0, 1, 3, 2, 4).reshape(N, BSH, CZ)
        parts.append(f)
    nq = np.concatenate(parts, axis=1) * np.float32(0.0625)   # (256, 512, 192)
    flatten = np.round(nq / q_zz)
    return (flatten, nq)


# revision 86
# speedup vs baseline: 1.1776x; 1.1776x over previous
"""JPEG encoder Bass kernel for TRN2 — self-contained, 8-core data-parallel.

kernel(img, D, Q) -> (flatten, no_quan_flatten), matching the reference:
    per 8x8 block: dct = D @ (X - 128) @ D.T ; quant = round(dct / Q);
    both zigzag-gathered + channel-concatenated to (256, 512, 192).

Device computes ONLY the unquantized DCT as int16 = round(16 * dct),
with X-128 pre-centered on the host (halves fp16 rounding noise; the
scale 16 is folded into the weights and |16*dct(X-128)| <= 16384 always
fits int16). The host derives nq = int16/16 and quant = round(nq/Q).
Dropping the on-device int8 quant stream cuts output DMA by 1/3 and
deletes the entire DVE quant pipeline.

Per 2-block-row strip (128 partitions = (brp, b), free = (c, i, w)):
  input DMA (2KB runs, SP queue) -> DVE regroup into z-grouped layout
  (c01 via two 4x-mode strided copies; c2 on gpsimd) -> 24 PE transposes
  in three 8-chunk units (c01 q0-3 | c2 all-q | c01 q4-7), each filling
  exactly one PSUM bank f16 -> DVE 2x copies to SBUF -> 24 single-shot
  fp16 matmuls against one block-diag(16*kron(D,D)[zz].T) weight
  (per-block c01 128-wide + merged c2 pair via the same block-diag) ->
  f32 PSUM half-strip tiles (3 banks, ring 2) -> Act f32->int16 cast
  copies (RNE) -> one whole-strip SWDGE out-DMA on the gpsimd queue
  (994ns fixed cost paid once; 6KB runs).

Schedule notes (cost-model driven): transposes are emitted two units
ahead (px ring 2 = 2 banks; po ring 2 = 6; PSUM exactly 8 banks); c01
matmuls are emitted a unit before the c2 matmuls that complete each po
half so the Act outcopy chain stays hot; DMA-issue SEQ-holds are kept
off the Act/DVE queues (inputs on SP, outputs on gpsimd/SWDGE whose
waits queue like normal engine ops); the last strip splits outcopies
Act||DVE and out-DMAs across two queues to shorten the drain. DMA floor
is ~35.3us/core (6.3MB fp16 in + 6.3MB int16 out at 360 GB/s).
"""

import numpy as np
import concourse.mybir as mybir
import concourse.tile as tile
from concourse import bacc
from concourse.bass_utils import run_bass_kernel_spmd

F32 = mybir.dt.float32
F16 = mybir.dt.float16
I16 = mybir.dt.int16
P = 8
B, C, H, W = 512, 3, 128, 128
NCORES = 8
BSH = B // NCORES          # 64 batches per core
N = (H // P) * (W // P)    # 256 blocks per plane
CZ = C * P * P             # 192
NBR = H // P               # 16 block rows
NSTRIP = NBR // 2          # 8 strips of 2 block rows

Copy = mybir.ActivationFunctionType.Copy


def _zigzag_flat_idx(n=P):
    order = []
    for s in range(2 * n - 1):
        cells = [(r, s - r) for r in range(max(0, s - n + 1), min(s, n - 1) + 1)]
        if s % 2 == 0:
            cells.reverse()
        order.extend(cells)
    return np.array([r * n + c for r, c in order], dtype=np.int32)


def _build_consts(D: np.ndarray):
    ZZ = _zigzag_flat_idx()
    D64 = D.astype(np.float64)
    KD = np.kron(D64, D64)[ZZ, :]          # (64 zz, 64 pix)
    Mt = (16.0 * KD.T).astype(np.float16)  # (pix, zz), x16 output scale
    Wm = np.zeros((128, 128), dtype=np.float16)
    Wm[0:64, 0:64] = Mt
    Wm[64:128, 64:128] = Mt
    bdid = np.eye(128, dtype=np.float16)
    return Wm, bdid


def _build_nc():
    nc = bacc.Bacc("TRN2", target_bir_lowering=False, debug=False)

    img = nc.dram_tensor("img", [BSH, C, H, W], F16, kind="ExternalInput")
    W_d = nc.dram_tensor("Wm", [128, 128], F16, kind="ExternalInput")
    # device layout: [strip, brp, b, (Q, c01|c2 packed)]; host reassembles
    out_d = nc.dram_tensor("out", [NSTRIP, 2, BSH, 3072], I16,
                           kind="ExternalOutput")

    # partition = (brp, b), free = (c, (i, w)) with 2KB contiguous runs
    imgv = img[:].rearrange("b c (bp brp i) w -> bp brp b c (i w)",
                            brp=2, i=P)

    with tile.TileContext(nc) as tc:
        with (
            tc.tile_pool(name="const", bufs=1) as constp,
            tc.tile_pool(name="sload", bufs=4) as sload,
            tc.tile_pool(name="greg", bufs=4) as greg,
            tc.tile_pool(name="x", bufs=6) as xp,
            tc.tile_pool(name="sb", bufs=5) as sbp,
            tc.tile_pool(name="sbl", bufs=4) as sblp,
            tc.tile_pool(name="px", bufs=2, space="PSUM") as pxp,
            tc.tile_pool(name="po", bufs=2, space="PSUM") as pop,
        ):
            Wm = constp.tile([128, 128], F16)
            bdid = constp.tile([128, 128], F16)
            ones = constp.tile([128, 128], F16)

            S_t = [None] * NSTRIP
            G_t = [None] * NSTRIP
            x_t = [None] * (NSTRIP * 4)
            px_t = [None] * (NSTRIP * 4)
            po_t = [None] * (NSTRIP * 2)
            sb_t = [None] * NSTRIP
            sbl_t = [None] * (NSTRIP * 2)

            def load_s(bp, split=False):
                S = sload.tile([128, 3072], F16, tag="s", name=f"s{bp}")
                if split:
                    # per-c-plane DMAs on three queues so the regroup
                    # copies can start as early as possible
                    Sv3 = S[:].rearrange("p (c iw) -> p c iw", c=3, iw=1024)
                    nc.sync.dma_start(out=Sv3[:, 0], in_=imgv[bp, :, :, 0])
                    nc.gpsimd.dma_start(out=Sv3[:, 1], in_=imgv[bp, :, :, 1])
                    nc.sync.dma_start(out=Sv3[:, 2], in_=imgv[bp, :, :, 2])
                else:
                    # per-plane DMAs: finer DMA_ENGINES granularity lets
                    # ready outputs interleave sooner than 2185ns strips
                    Sv3 = S[:].rearrange("p (c iw) -> p c iw", c=3, iw=1024)
                    for c in range(3):
                        nc.sync.dma_start(out=Sv3[:, c],
                                          in_=imgv[bp, :, :, c])
                S_t[bp] = S

            def regroup(bp, part=None):
                # S free (c,i,q,k,j) -> G (q, z, i, j); z = 2k+c | 4+k
                # part 0/1: c01 k-halves; part 2: c2 (emitted first).
                if part in (2, None):
                    G_t[bp] = greg.tile([128, 3072], F16, tag="g",
                                        name=f"g{bp}")
                G = G_t[bp]
                gv = G[:].rearrange("p (q z i j) -> p z q i j",
                                    q=8, z=6, i=P, j=P)
                sv = S_t[bp][:].rearrange("p (c i q k j) -> p k c q i j",
                                          c=3, i=P, q=8, k=2, j=P)
                parts = range(3) if part is None else [part]
                for pt in parts:
                    if pt < 2:
                        nc.vector.tensor_copy(gv[:, 2 * pt:2 * pt + 2],
                                              sv[:, pt, 0:2])
                    elif part is None:
                        nc.vector.tensor_copy(gv[:, 4:6], sv[:, :, 2])
                    else:
                        # c2 regroup on gpsimd, emitted ahead of this
                        # strip's out-DMAs in the gpsimd queue
                        nc.gpsimd.tensor_copy(gv[:, 4:6], sv[:, :, 2])

            def transposes(v):
                # unit a (uu=0): c01 of q0-3; unit b (uu=1): c2 of all q;
                # unit c (uu=2): c01 of q4-7. Each 8 chunks = 1 PSUM bank.
                bp, uu = v // 3, v % 3
                G = G_t[bp]
                px = pxp.tile([128, 1024], F16, tag="px", name=f"px{v}")
                if uu == 1:
                    for q in range(8):
                        nc.tensor.matmul(
                            px[:, q * 128:(q + 1) * 128],
                            G[:, q * 384 + 256:q * 384 + 384],
                            bdid[:], is_transpose=True)
                else:
                    q0 = 0 if uu == 0 else 4
                    for t in range(4):
                        q = q0 + t
                        for k in range(2):
                            c = t * 2 + k
                            nc.tensor.matmul(
                                px[:, c * 128:(c + 1) * 128],
                                G[:, q * 384 + k * 128:
                                  q * 384 + (k + 1) * 128],
                                bdid[:], is_transpose=True)
                px_t[v] = px

            def xcopy(v):
                uu = v % 3
                x = xp.tile([128, 1024], F16, tag="x", name=f"x{v}")
                if uu == 1:
                    # split so M(h0) waits only the first c2 half
                    nc.vector.tensor_copy(x[:, 0:512], px_t[v][:, 0:512])
                    nc.vector.tensor_copy(x[:, 512:1024],
                                          px_t[v][:, 512:1024])
                else:
                    nc.vector.tensor_copy(x[:], px_t[v][:])
                x_t[v] = x

            def mm_c01(h):
                # half-strip po tile: [c01 8x128 | c2 4x128] per half
                bp, hh = h // 2, h % 2
                xa = x_t[bp * 3 + (0 if hh == 0 else 2)]
                if hh == 0:
                    po_t[h] = pop.tile([128, 1536], F32, tag="po",
                                       name=f"po{h}")
                po = po_t[h]
                for t in range(8):
                    nc.tensor.matmul(po[:, t * 128:(t + 1) * 128],
                                     xa[:, t * 128:(t + 1) * 128], Wm[:],
                                     start=True, stop=True)

            def mm_c2(h):
                bp, hh = h // 2, h % 2
                xb = x_t[bp * 3 + 1]
                if hh == 1:
                    po_t[h] = pop.tile([128, 1536], F32, tag="po",
                                       name=f"po{h}")
                po = po_t[h]
                for p in range(4):
                    nc.tensor.matmul(
                        po[:, 1024 + p * 128:1024 + (p + 1) * 128],
                        xb[:, (hh * 4 + p) * 128:(hh * 4 + p + 1) * 128],
                        Wm[:], start=True, stop=True)

            def outcopy(h, last=False):
                bp, hh = h // 2, h % 2
                if hh == 0 and not last:
                    sb_t[bp] = sbp.tile([128, 3072], I16, tag="sb",
                                        name=f"sb{bp}")
                if last:
                    # last strip: separate tiles per piece so the Act/DVE
                    # split copies aren't serialized by same-tile WAW deps
                    ta = sblp.tile([128, 768], I16, tag="sbl",
                                   name=f"sbl{h}a")
                    tb = sblp.tile([128, 768], I16, tag="sbl",
                                   name=f"sbl{h}b")
                    nc.scalar.activation(ta[:], po_t[h][:, 0:768], Copy)
                    nc.vector.tensor_copy(tb[:], po_t[h][:, 768:1536])
                    sbl_t[h] = (ta, tb)
                    return
                dst = sb_t[bp][:, hh * 1536:(hh + 1) * 1536]
                nc.scalar.activation(dst, po_t[h][:], Copy)

            def out_dma(bp, half, last=False):
                base = half * 1536
                if last:
                    # last strip: two parallel-queue DMAs to shorten tail
                    ta, tb = sbl_t[bp * 2 + half]
                    q2 = nc.sync if half else nc.scalar
                    nc.gpsimd.dma_start(
                        out=out_d[bp, :, :, base:base + 768], in_=ta[:])
                    q2.dma_start(
                        out=out_d[bp, :, :, base + 768:base + 1536],
                        in_=tb[:])
                    return
                fsl = slice(base, base + 1536)
                nc.gpsimd.dma_start(out=out_d[bp, :, :, fsl],
                                    in_=sb_t[bp][:, fsl])

            # ---- software-pipelined emission over 24 units ----
            # build the transpose identity on-device (memset + affine
            # select on iota p-f == 0): ready ~0.5us, no DMA in the lead
            # FIFO; Wm rides the SWDGE queue so it can't delay the planes
            load_s(0, split=True)
            nc.gpsimd.memset(ones[:], 1.0)
            nc.gpsimd.affine_select(bdid[:], ones[:], pattern=[[-1, 128]],
                                    compare_op=mybir.AluOpType.is_equal,
                                    fill=0.0, base=0, channel_multiplier=1)
            nc.gpsimd.dma_start(out=Wm[:], in_=W_d[:])
            regroup(0)
            load_s(1)
            # PE p-state warmup: dummy transposes on bdid bridge the gap
            # until the first real transpose so the 3us ramp to full clock
            # starts at ~3us instead of ~5us
            warm = pxp.tile([128, 1024], F16, tag="px", name="warm")
            for w in range(16):
                nc.tensor.matmul(warm[:, (w % 8) * 128:(w % 8 + 1) * 128],
                                 bdid[:], bdid[:], is_transpose=True)
            transposes(0)
            transposes(1)
            NV = NSTRIP * 3
            for v in range(NV):
                bp, uu = v // 3, v % 3
                if uu == 2 and bp + 2 < NSTRIP:
                    load_s(bp + 2)
                xcopy(v)
                if bp + 1 < NSTRIP:
                    # uu0 -> c2 on gpsimd (ahead of this strip's out-DMAs
                    # in the gpsimd queue) + c01-k0; uu1 -> c01-k1
                    if uu == 0:
                        regroup(bp + 1, part=2)
                        regroup(bp + 1, part=0)
                    elif uu == 1:
                        regroup(bp + 1, part=1)
                if uu == 0:
                    mm_c01(bp * 2)
                elif uu == 1:
                    mm_c2(bp * 2)
                else:
                    mm_c01(bp * 2 + 1)
                if v + 2 < NV:
                    transposes(v + 2)
                if uu == 1:
                    # po(h1) alloc waits the ring; emit after T so the
                    # wait can't head-of-line-block the transposes
                    mm_c2(bp * 2 + 1)
                if uu > 0:
                    h = bp * 2 + uu - 1
                    last = bp == NSTRIP - 1
                    outcopy(h, last=last)
                    # one whole-strip out-DMA (halves the SWDGE fixed
                    # cost); the last strip keeps per-half split DMAs
                    if last:
                        out_dma(bp, h % 2, last=True)
                    elif bp >= NSTRIP - 3:
                        # drain phase: per-half DMAs so h0 transfers early
                        out_dma(bp, h % 2)
                    elif uu == 2:
                        nc.gpsimd.dma_start(out=out_d[bp],
                                            in_=sb_t[bp][:])

    nc.compile()
    return nc


_NC_CACHE = None


def _get_nc():
    global _NC_CACHE
    if _NC_CACHE is None:
        _NC_CACHE = _build_nc()
    return _NC_CACHE


def _build_perm():
    # device free offset for (bw, czz): half h = bw//8, local bwl = bw%8;
    # c01 at h*1536 + bwl*128 + c*64 + zz; c2 at h*1536 + 1024 + bwl*64 + zz
    perm = np.zeros(NBR * CZ, dtype=np.int64)
    for bw in range(16):
        h, bwl = bw // 8, bw % 8
        for c in range(3):
            for zz in range(64):
                col = bw * CZ + c * 64 + zz
                if c < 2:
                    off = h * 1536 + bwl * 128 + c * 64 + zz
                else:
                    off = h * 1536 + 1024 + bwl * 64 + zz
                perm[col] = off
    return perm


_PERM = _build_perm()


def kernel(img, D, Q):
    img = np.asarray(img, dtype=np.float32)
    D = np.asarray(D, dtype=np.float32)
    Q = np.asarray(Q, dtype=np.float32)
    Wm, _ = _build_consts(D)
    ZZ = _zigzag_flat_idx()
    q_zz = np.tile(Q.flatten()[ZZ], C).astype(np.float32)     # (192,)

    # subtract 128 on host: halves fp16 input/weight noise and keeps
    # |16*dct| <= 16384 in int16 (dct of X-128 matches the reference)
    img16 = np.ascontiguousarray(img - np.float32(128.0)).astype(np.float16)
    nc = _get_nc()
    in_maps = [
        {"img": img16[kk * BSH:(kk + 1) * BSH], "Wm": Wm}
        for kk in range(NCORES)
    ]
    res = run_bass_kernel_spmd(nc, in_maps, core_ids=list(range(NCORES)))

    parts = []
    for r in res.results:
        dev = np.asarray(r["out"])                 # (8, 2, 64, 3072) i16
        f = dev[..., _PERM].astype(np.float32)     # (8, 2, 64, 16*192)
        f = f.reshape(NSTRIP, 2, BSH, NBR, CZ)
        f = f.transpose(0, 1, 3, 2, 4).reshape(N, BSH, CZ)
        parts.append(f)
    nq = np.concatenate(parts, axis=1) * np.float32(0.0625)   # (256, 512, 192)
    flatten = np.round(nq / q_zz)
    return (flatten, nq)


# revision 87
# speedup vs baseline: 1.1990x; 1.0182x over previous
"""JPEG encoder Bass kernel for TRN2 — self-contained, 8-core data-parallel.

kernel(img, D, Q) -> (flatten, no_quan_flatten), matching the reference:
    per 8x8 block: dct = D @ (X - 128) @ D.T ; quant = round(dct / Q);
    both zigzag-gathered + channel-concatenated to (256, 512, 192).

Device computes ONLY the unquantized DCT as int16 = round(16 * dct),
with X-128 pre-centered on the host (halves fp16 rounding noise; the
scale 16 is folded into the weights and |16*dct(X-128)| <= 16384 always
fits int16). The host derives nq = int16/16 and quant = round(nq/Q).
Dropping the on-device int8 quant stream cuts output DMA by 1/3 and
deletes the entire DVE quant pipeline.

Per 2-block-row strip (128 partitions = (brp, b), free = (c, i, w)):
  input DMA (2KB runs, SP queue) -> DVE regroup into z-grouped layout
  (c01 via two 4x-mode strided copies; c2 on gpsimd) -> 24 PE transposes
  in three 8-chunk units (c01 q0-3 | c2 all-q | c01 q4-7), each filling
  exactly one PSUM bank f16 -> DVE 2x copies to SBUF -> 24 single-shot
  fp16 matmuls against one block-diag(16*kron(D,D)[zz].T) weight
  (per-block c01 128-wide + merged c2 pair via the same block-diag) ->
  f32 PSUM half-strip tiles (3 banks, ring 2) -> Act f32->int16 cast
  copies (RNE) -> one whole-strip SWDGE out-DMA on the gpsimd queue
  (994ns fixed cost paid once; 6KB runs).

Schedule notes (cost-model driven): transposes are emitted two units
ahead (px ring 2 = 2 banks; po ring 2 = 6; PSUM exactly 8 banks); c01
matmuls are emitted a unit before the c2 matmuls that complete each po
half so the Act outcopy chain stays hot; DMA-issue SEQ-holds are kept
off the Act/DVE queues (inputs on SP, outputs on gpsimd/SWDGE whose
waits queue like normal engine ops); the last strip splits outcopies
Act||DVE and out-DMAs across two queues to shorten the drain. DMA floor
is ~35.3us/core (6.3MB fp16 in + 6.3MB int16 out at 360 GB/s).
"""

import numpy as np
import concourse.mybir as mybir
import concourse.tile as tile
from concourse import bacc
from concourse.bass_utils import run_bass_kernel_spmd

F32 = mybir.dt.float32
F16 = mybir.dt.float16
I16 = mybir.dt.int16
P = 8
B, C, H, W = 512, 3, 128, 128
NCORES = 8
BSH = B // NCORES          # 64 batches per core
N = (H // P) * (W // P)    # 256 blocks per plane
CZ = C * P * P             # 192
NBR = H // P               # 16 block rows
NSTRIP = NBR // 2          # 8 strips of 2 block rows

Copy = mybir.ActivationFunctionType.Copy


def _zigzag_flat_idx(n=P):
    order = []
    for s in range(2 * n - 1):
        cells = [(r, s - r) for r in range(max(0, s - n + 1), min(s, n - 1) + 1)]
        if s % 2 == 0:
            cells.reverse()
        order.extend(cells)
    return np.array([r * n + c for r, c in order], dtype=np.int32)


def _build_consts(D: np.ndarray):
    ZZ = _zigzag_flat_idx()
    D64 = D.astype(np.float64)
    KD = np.kron(D64, D64)[ZZ, :]          # (64 zz, 64 pix)
    Mt = (16.0 * KD.T).astype(np.float16)  # (pix, zz), x16 output scale
    Wm = np.zeros((128, 128), dtype=np.float16)
    Wm[0:64, 0:64] = Mt
    Wm[64:128, 64:128] = Mt
    bdid = np.eye(128, dtype=np.float16)
    return Wm, bdid


def _build_nc():
    nc = bacc.Bacc("TRN2", target_bir_lowering=False, debug=False)

    img = nc.dram_tensor("img", [BSH, C, H, W], F16, kind="ExternalInput")
    W_d = nc.dram_tensor("Wm", [128, 128], F16, kind="ExternalInput")
    # device layout: [strip, brp, b, (Q, c01|c2 packed)]; host reassembles
    out_d = nc.dram_tensor("out", [NSTRIP, 2, BSH, 3072], I16,
                           kind="ExternalOutput")

    # partition = (brp, b), free = (c, (i, w)) with 2KB contiguous runs
    imgv = img[:].rearrange("b c (bp brp i) w -> bp brp b c (i w)",
                            brp=2, i=P)

    with tile.TileContext(nc) as tc:
        with (
            tc.tile_pool(name="const", bufs=1) as constp,
            tc.tile_pool(name="sload", bufs=4) as sload,
            tc.tile_pool(name="greg", bufs=4) as greg,
            tc.tile_pool(name="x", bufs=6) as xp,
            tc.tile_pool(name="sb", bufs=5) as sbp,
            tc.tile_pool(name="sbl", bufs=4) as sblp,
            tc.tile_pool(name="px", bufs=2, space="PSUM") as pxp,
            tc.tile_pool(name="po", bufs=2, space="PSUM") as pop,
        ):
            Wm = constp.tile([128, 128], F16)
            bdid = constp.tile([128, 128], F16)
            ones = constp.tile([128, 128], F16)

            S_t = [None] * NSTRIP
            G_t = [None] * NSTRIP
            x_t = [None] * (NSTRIP * 4)
            px_t = [None] * (NSTRIP * 4)
            po_t = [None] * (NSTRIP * 2)
            sb_t = [None] * NSTRIP
            sbl_t = [None] * (NSTRIP * 2)

            def load_s(bp, split=False):
                S = sload.tile([128, 3072], F16, tag="s", name=f"s{bp}")
                if split:
                    # per-c-plane DMAs on three queues so the regroup
                    # copies can start as early as possible
                    Sv3 = S[:].rearrange("p (c iw) -> p c iw", c=3, iw=1024)
                    nc.sync.dma_start(out=Sv3[:, 0], in_=imgv[bp, :, :, 0])
                    nc.gpsimd.dma_start(out=Sv3[:, 1], in_=imgv[bp, :, :, 1])
                    nc.sync.dma_start(out=Sv3[:, 2], in_=imgv[bp, :, :, 2])
                else:
                    # per-plane DMAs: finer DMA_ENGINES granularity lets
                    # ready outputs interleave sooner than 2185ns strips
                    Sv3 = S[:].rearrange("p (c iw) -> p c iw", c=3, iw=1024)
                    for c in range(3):
                        nc.sync.dma_start(out=Sv3[:, c],
                                          in_=imgv[bp, :, :, c])
                S_t[bp] = S

            def regroup(bp, part=None):
                # S free (c,i,q,k,j) -> G (q, z, i, j); z = 2k+c | 4+k
                # part 0/1: c01 k-halves; part 2: c2 (emitted first).
                if part in (2, None):
                    G_t[bp] = greg.tile([128, 3072], F16, tag="g",
                                        name=f"g{bp}")
                G = G_t[bp]
                gv = G[:].rearrange("p (q z i j) -> p z q i j",
                                    q=8, z=6, i=P, j=P)
                sv = S_t[bp][:].rearrange("p (c i q k j) -> p k c q i j",
                                          c=3, i=P, q=8, k=2, j=P)
                parts = range(3) if part is None else [part]
                for pt in parts:
                    if pt < 2:
                        nc.vector.tensor_copy(gv[:, 2 * pt:2 * pt + 2],
                                              sv[:, pt, 0:2])
                    elif part is None:
                        nc.vector.tensor_copy(gv[:, 4:6], sv[:, :, 2])
                    else:
                        # c2 regroup on gpsimd, emitted ahead of this
                        # strip's out-DMAs in the gpsimd queue
                        nc.gpsimd.tensor_copy(gv[:, 4:6], sv[:, :, 2])

            def transposes(v):
                # unit a (uu=0): c01 of q0-3; unit b (uu=1): c2 of all q;
                # unit c (uu=2): c01 of q4-7. Each 8 chunks = 1 PSUM bank.
                bp, uu = v // 3, v % 3
                G = G_t[bp]
                px = pxp.tile([128, 1024], F16, tag="px", name=f"px{v}")
                if uu == 1:
                    for q in range(8):
                        nc.tensor.matmul(
                            px[:, q * 128:(q + 1) * 128],
                            G[:, q * 384 + 256:q * 384 + 384],
                            bdid[:], is_transpose=True)
                else:
                    q0 = 0 if uu == 0 else 4
                    for t in range(4):
                        q = q0 + t
                        for k in range(2):
                            c = t * 2 + k
                            nc.tensor.matmul(
                                px[:, c * 128:(c + 1) * 128],
                                G[:, q * 384 + k * 128:
                                  q * 384 + (k + 1) * 128],
                                bdid[:], is_transpose=True)
                px_t[v] = px

            def xcopy(v):
                uu = v % 3
                x = xp.tile([128, 1024], F16, tag="x", name=f"x{v}")
                if uu == 1:
                    # split so M(h0) waits only the first c2 half
                    nc.vector.tensor_copy(x[:, 0:512], px_t[v][:, 0:512])
                    nc.vector.tensor_copy(x[:, 512:1024],
                                          px_t[v][:, 512:1024])
                else:
                    nc.vector.tensor_copy(x[:], px_t[v][:])
                x_t[v] = x

            def mm_c01(h):
                # half-strip po tile: [c01 8x128 | c2 4x128] per half
                bp, hh = h // 2, h % 2
                xa = x_t[bp * 3 + (0 if hh == 0 else 2)]
                if hh == 0:
                    po_t[h] = pop.tile([128, 1536], F32, tag="po",
                                       name=f"po{h}")
                po = po_t[h]
                for t in range(8):
                    nc.tensor.matmul(po[:, t * 128:(t + 1) * 128],
                                     xa[:, t * 128:(t + 1) * 128], Wm[:],
                                     start=True, stop=True)

            def mm_c2(h):
                bp, hh = h // 2, h % 2
                xb = x_t[bp * 3 + 1]
                if hh == 1:
                    po_t[h] = pop.tile([128, 1536], F32, tag="po",
                                       name=f"po{h}")
                po = po_t[h]
                for p in range(4):
                    nc.tensor.matmul(
                        po[:, 1024 + p * 128:1024 + (p + 1) * 128],
                        xb[:, (hh * 4 + p) * 128:(hh * 4 + p + 1) * 128],
                        Wm[:], start=True, stop=True)

            def outcopy(h, last=False):
                bp, hh = h // 2, h % 2
                if hh == 0 and not last:
                    sb_t[bp] = sbp.tile([128, 3072], I16, tag="sb",
                                        name=f"sb{bp}")
                if last:
                    # last strip: separate tiles per piece so the Act/DVE
                    # split copies aren't serialized by same-tile WAW deps
                    ta = sblp.tile([128, 768], I16, tag="sbl",
                                   name=f"sbl{h}a")
                    tb = sblp.tile([128, 768], I16, tag="sbl",
                                   name=f"sbl{h}b")
                    nc.scalar.activation(ta[:], po_t[h][:, 0:768], Copy)
                    nc.vector.tensor_copy(tb[:], po_t[h][:, 768:1536])
                    sbl_t[h] = (ta, tb)
                    return
                dst = sb_t[bp][:, hh * 1536:(hh + 1) * 1536]
                nc.scalar.activation(dst, po_t[h][:], Copy)

            def out_dma(bp, half, last=False):
                base = half * 1536
                if last:
                    # last strip: two parallel-queue DMAs to shorten tail
                    ta, tb = sbl_t[bp * 2 + half]
                    q2 = nc.sync if half else nc.scalar
                    nc.gpsimd.dma_start(
                        out=out_d[bp, :, :, base:base + 768], in_=ta[:])
                    q2.dma_start(
                        out=out_d[bp, :, :, base + 768:base + 1536],
                        in_=tb[:])
                    return
                fsl = slice(base, base + 1536)
                nc.gpsimd.dma_start(out=out_d[bp, :, :, fsl],
                                    in_=sb_t[bp][:, fsl])

            # ---- software-pipelined emission over 24 units ----
            # build the transpose identity on-device (memset + affine
            # select on iota p-f == 0): ready ~0.5us, no DMA in the lead
            # FIFO; Wm rides the SWDGE queue so it can't delay the planes
            load_s(0, split=True)
            nc.gpsimd.memset(ones[:], 1.0)
            nc.gpsimd.affine_select(bdid[:], ones[:], pattern=[[-1, 128]],
                                    compare_op=mybir.AluOpType.is_equal,
                                    fill=0.0, base=0, channel_multiplier=1)
            nc.gpsimd.dma_start(out=Wm[:], in_=W_d[:])
            regroup(0)
            load_s(1)
            # PE p-state warmup: dummy transposes on bdid bridge the gap
            # until the first real transpose so the 3us ramp to full clock
            # starts at ~3us instead of ~5us
            warm = pxp.tile([128, 1024], F16, tag="px", name="warm")
            for w in range(16):
                nc.tensor.matmul(warm[:, (w % 8) * 128:(w % 8 + 1) * 128],
                                 bdid[:], bdid[:], is_transpose=True)
            transposes(0)
            transposes(1)
            NV = NSTRIP * 3
            for v in range(NV):
                bp, uu = v // 3, v % 3
                if uu == 2 and bp + 2 < NSTRIP:
                    load_s(bp + 2)
                xcopy(v)
                if bp + 1 < NSTRIP:
                    # uu0 -> c2 on gpsimd (ahead of this strip's out-DMAs
                    # in the gpsimd queue) + c01-k0; uu1 -> c01-k1
                    if uu == 0:
                        regroup(bp + 1, part=2)
                        regroup(bp + 1, part=0)
                    elif uu == 1:
                        regroup(bp + 1, part=1)
                if uu == 0:
                    mm_c01(bp * 2)
                elif uu == 1:
                    mm_c2(bp * 2)
                else:
                    mm_c01(bp * 2 + 1)
                if v + 2 < NV:
                    transposes(v + 2)
                if uu == 1:
                    # po(h1) alloc waits the ring; emit after T so the
                    # wait can't head-of-line-block the transposes
                    mm_c2(bp * 2 + 1)
                if uu == 0 and 0 < bp <= NSTRIP - 3:
                    # whole-strip out-DMA for bp-1, emitted AFTER this
                    # strip's regroup-C so the gen's outcopy-wait doesn't
                    # delay C on the in-order gpsimd engine
                    nc.gpsimd.dma_start(out=out_d[bp - 1],
                                        in_=sb_t[bp - 1][:])
                if uu > 0:
                    h = bp * 2 + uu - 1
                    last = bp == NSTRIP - 1
                    outcopy(h, last=last)
                    # the last strip keeps per-half split DMAs
                    if last:
                        out_dma(bp, h % 2, last=True)
                    elif bp >= NSTRIP - 3:
                        # drain phase: per-half DMAs so h0 transfers early
                        out_dma(bp, h % 2)


    nc.compile()
    return nc


_NC_CACHE = None


def _get_nc():
    global _NC_CACHE
    if _NC_CACHE is None:
        _NC_CACHE = _build_nc()
    return _NC_CACHE


def _build_perm():
    # device free offset for (bw, czz): half h = bw//8, local bwl = bw%8;
    # c01 at h*1536 + bwl*128 + c*64 + zz; c2 at h*1536 + 1024 + bwl*64 + zz
    perm = np.zeros(NBR * CZ, dtype=np.int64)
    for bw in range(16):
        h, bwl = bw // 8, bw % 8
        for c in range(3):
            for zz in range(64):
                col = bw * CZ + c * 64 + zz
                if c < 2:
                    off = h * 1536 + bwl * 128 + c * 64 + zz
                else:
                    off = h * 1536 + 1024 + bwl * 64 + zz
                perm[col] = off
    return perm


_PERM = _build_perm()


def kernel(img, D, Q):
    img = np.asarray(img, dtype=np.float32)
    D = np.asarray(D, dtype=np.float32)
    Q = np.asarray(Q, dtype=np.float32)
    Wm, _ = _build_consts(D)
    ZZ = _zigzag_flat_idx()
    q_zz = np.tile(Q.flatten()[ZZ], C).astype(np.float32)     # (192,)

    # subtract 128 on host: halves fp16 input/weight noise and keeps
    # |16*dct| <= 16384 in int16 (dct of X-128 matches the reference)
    img16 = np.ascontiguousarray(img - np.float32(128.0)).astype(np.float16)
    nc = _get_nc()
    in_maps = [
        {"img": img16[kk * BSH:(kk + 1) * BSH], "Wm": Wm}
        for kk in range(NCORES)
    ]
    res = run_bass_kernel_spmd(nc, in_maps, core_ids=list(range(NCORES)))

    parts = []
    for r in res.results:
        dev = np.asarray(r["out"])                 # (8, 2, 64, 3072) i16
        f = dev[..., _PERM].astype(np.float32)     # (8, 2, 64, 16*192)
        f = f.reshape(NSTRIP, 2, BSH, NBR, CZ)
        f = f.transpose(0, 1, 3, 2, 4).reshape(N, BSH, CZ)
        parts.append(f)
    nq = np.concatenate(parts, axis=1) * np.float32(0.0625)   # (256, 512, 192)
    flatten = np.round(nq / q_zz)
    return (flatten, nq)
